# revision 1
# baseline (speedup 1.0000x reference)
"""LongRangeProj Bass kernel for TRN2 (8 NeuronCores, channel-sharded).

Math: out[b,c,h,w] = max_{o=(i,j)} x[b,c,o] * exp(-(inv2rv*(fn-|rm|)^2
                                                   + inv2av*wrap(theta-a)^2))
with fn/theta = polar coords of pixel (h,w) around origin o, and the angle
term forced to 1 at the origin pixel itself (mask).  exp is monotone, so the
max is taken on the exponent and exp applied to the reduced [B,C,H,W] only.

Per-core layout: partitions = 2 batches x 64 origins, free dim = 4096 pixels.
Each core owns C/8 = 8 channels; one channel per iteration.
Engines: ACT (affine+Square+Exp, one table set), DVE (sub/add + PSUM max
reduce), GPSIMD (round-trick + mask mul), PE (128x128 fp32 transposes).
"""

import numpy as np
from contextlib import ExitStack

B, C, NH, NW, H, W = 2, 64, 8, 8, 64, 64
STRIDE = 8
NCORES = 8
CL = C // NCORES          # channels per core
HW = H * W                # 4096
NO = NH * NW              # 64 origins
FREE_CHUNK = 2048
NBLK = HW // 128          # 32 pixel blocks of 128
CBIG = float(1.5 * 2 ** 23)   # fp32 round-to-nearest magic constant
TWO_PI = 2.0 * np.pi

_built = {}


def _host_fields():
    """Constant geometric fields in [NO, HW] layout, fp32."""
    oy = np.arange(NH, dtype=np.float64) * STRIDE
    ox = np.arange(NW, dtype=np.float64) * STRIDE
    yg = np.arange(H, dtype=np.float64)
    xg = np.arange(W, dtype=np.float64)
    fy = yg[None, :] - oy[:, None]                      # [NH, H]
    fx = xg[None, :] - ox[:, None]                      # [NW, W]
    FY = np.broadcast_to(fy[:, None, :, None], (NH, NW, H, W))
    FX = np.broadcast_to(fx[None, :, None, :], (NH, NW, H, W))
    fn = np.sqrt(FX * FX + FY * FY)
    theta = np.arctan2(FY, FX)
    v = theta / TWO_PI
    mask = np.zeros((NH, NW, H, W), dtype=np.float64)
    for i in range(NH):
        for j in range(NW):
            mask[i, j, i * STRIDE, j * STRIDE] = 1.0
    notm = 1.0 - mask
    rs = lambda a: np.ascontiguousarray(a.reshape(NO, HW).astype(np.float32))
    return rs(v), rs(fn), rs(notm)


def _build_bass():
    import concourse.bass as bass
    import concourse.bacc as bacc
    import concourse.tile as tile
    import concourse.mybir as mybir

    f32 = mybir.dt.float32
    AF = mybir.ActivationFunctionType
    OP = mybir.AluOpType
    AX = mybir.AxisListType

    CW = 3 * HW + 128 + 5 * CL   # packed const width
    nc = bacc.Bacc("TRN2", target_bir_lowering=False)
    cst_d = nc.dram_tensor("cst", [128, CW], f32, kind="ExternalInput")
    out_d = nc.dram_tensor("out", [2 * CL, HW], f32, kind="ExternalOutput")

    with ExitStack() as ctx:
        tc = ctx.enter_context(tile.TileContext(nc))
        cpool = ctx.enter_context(tc.tile_pool(name="const", bufs=1))
        work = ctx.enter_context(tc.tile_pool(name="work", bufs=2))
        psum = ctx.enter_context(tc.tile_pool(name="psum", bufs=8, space="PSUM"))
        outp = ctx.enter_context(tc.tile_pool(name="outp", bufs=2))

        CST = cpool.tile([128, CW], f32, tag="CST")
        nc.gpsimd.dma_start(CST[:, :], cst_d[:, :])
        V = CST[:, 0:HW]
        FNT = CST[:, HW : 2 * HW]
        NM = CST[:, 2 * HW : 3 * HW]
        ID = CST[:, 3 * HW : 3 * HW + 128]
        SCAL = CST[:, 3 * HW + 128 :]
        A2 = SCAL[:, 0 * CL : 1 * CL]
        S2 = SCAL[:, 1 * CL : 2 * CL]
        SR = SCAL[:, 2 * CL : 3 * CL]
        BR = SCAL[:, 3 * CL : 4 * CL]
        LX = SCAL[:, 4 * CL : 5 * CL]

        nchunk = HW // FREE_CHUNK
        blk_per_chunk = FREE_CHUNK // 128          # 16
        grp_per_chunk = blk_per_chunk // 4         # 4 (one PSUM bank each)

        for it in range(CL):
            a2 = A2[:, it : it + 1]
            s2 = S2[:, it : it + 1]
            sr = SR[:, it : it + 1]
            br = BR[:, it : it + 1]
            lx = LX[:, it : it + 1]
            o_t = outp.tile([128, NBLK, 2], f32, tag="o_t")
            for ch in range(nchunk):
                sl = slice(ch * FREE_CHUNK, (ch + 1) * FREE_CHUNK)
                # u = theta/2pi - a/2pi
                u = work.tile([128, FREE_CHUNK], f32, tag="u")
                nc.scalar.activation(u[:], V[:, sl], AF.Identity, bias=a2)
                # rr = round(u)  via (u + C) - C
                rr = work.tile([128, FREE_CHUNK], f32, tag="rr")
                nc.gpsimd.tensor_scalar(rr[:], u[:], CBIG, CBIG, OP.add, OP.subtract)
                # wu = u - round(u)  in [-0.5, 0.5]
                wu = work.tile([128, FREE_CHUNK], f32, tag="wu")
                nc.vector.tensor_tensor(wu[:], u[:], rr[:], OP.subtract)
                # mask: zero the angle at each origin's own pixel
                wm = work.tile([128, FREE_CHUNK], f32, tag="wm")
                nc.gpsimd.tensor_tensor(wm[:], wu[:], NM[:, sl], OP.mult)
                # sqa = (2pi*sqrt(inv2av) * wm)^2
                sqa = work.tile([128, FREE_CHUNK], f32, tag="sqa")
                nc.scalar.activation(sqa[:], wm[:], AF.Square, scale=s2)
                # rdn = (sqrt(inv2rv)*fn - rm*sqrt(inv2rv))^2
                rdn = work.tile([128, FREE_CHUNK], f32, tag="rdn")
                nc.scalar.activation(rdn[:], FNT[:, sl], AF.Square, scale=sr, bias=br)
                # t = sqa + rdn ; s = -t + ln x
                tt = work.tile([128, FREE_CHUNK], f32, tag="tt")
                nc.vector.tensor_tensor(tt[:], sqa[:], rdn[:], OP.add)
                s = work.tile([128, FREE_CHUNK], f32, tag="s")
                nc.scalar.activation(s[:], tt[:], AF.Identity, scale=-1.0, bias=lx)
                # transpose 128x128 blocks to PSUM, max-reduce origins
                for g in range(grp_per_chunk):
                    ps = psum.tile([128, 512], f32, tag="ps")
                    for l in range(4):
                        nc.tensor.transpose(
                            ps[:, l * 128 : (l + 1) * 128],
                            s[:, (g * 4 + l) * 128 : (g * 4 + l + 1) * 128],
                            ID[:, :],
                        )
                    red_in = ps[:, :].rearrange("p (l r o) -> p l r o", l=4, r=2, o=64)
                    b0 = ch * blk_per_chunk + g * 4
                    nc.vector.tensor_reduce(
                        o_t[:, b0 : b0 + 4, :], red_in, axis=AX.X, op=OP.max
                    )
            o_e = outp.tile([128, NBLK, 2], f32, tag="o_e")
            nc.scalar.activation(o_e[:, :, :], o_t[:, :, :], AF.Exp)
            for pair in range(2):
                row = pair * CL + it
                nc.sync.dma_start(
                    out_d[row].rearrange("(blk p) -> p blk", p=128),
                    o_e[:, :, pair],
                )
    nc.finalize()
    return nc


def _host_scalars(x, radius_mean, angle_mean, radius_std, angle_std):
    """Per-core scalar tables [128, CL], fp64->fp32. partition = b*64 + o."""
    inv2rv = 1.0 / (2.0 * (radius_std.astype(np.float64) ** 2 + 0.01))   # [C]
    inv2av = 1.0 / (2.0 * (angle_std.astype(np.float64) ** 2 + 0.0001))  # [C]
    rm = np.abs(radius_mean.astype(np.float64)).reshape(B, C, NO)
    am = angle_mean.astype(np.float64).reshape(B, C, NO)
    xx = np.maximum(x.astype(np.float64).reshape(B, C, NO), 1e-30)
    per_core = []
    for k in range(NCORES):
        cs = np.arange(k * CL, (k + 1) * CL)
        a2 = np.zeros((128, CL)); s2 = np.zeros((128, CL))
        sr = np.zeros((128, CL)); br = np.zeros((128, CL))
        lxv = np.zeros((128, CL))
        for itc, c in enumerate(cs):
            srt = np.sqrt(inv2rv[c])
            for b in range(B):
                p = slice(b * NO, (b + 1) * NO)
                a2[p, itc] = -am[b, c] / TWO_PI
                s2[p, itc] = TWO_PI * np.sqrt(inv2av[c])
                sr[p, itc] = srt
                br[p, itc] = -rm[b, c] * srt
                lxv[p, itc] = np.log(xx[b, c])
        f = lambda a: np.ascontiguousarray(a.astype(np.float32))
        per_core.append(dict(a2=f(a2), s2=f(s2), sr=f(sr), br=f(br), lx=f(lxv)))
    return per_core


def kernel(x, radius_mean, angle_mean, radius_std, angle_std):
    from concourse.bass_utils import run_bass_kernel_spmd

    if "nc" not in _built:
        _built["nc"] = _build_bass()
        _built["fields"] = _host_fields()
    nc = _built["nc"]
    v, fn, nm = _built["fields"]
    fld = np.concatenate([v, fn, nm], axis=1)          # [64, 3*HW]
    fld2 = np.concatenate([fld, fld], axis=0)          # [128, 3*HW]
    ident = np.eye(128, dtype=np.float32)
    sc = _host_scalars(x, radius_mean, angle_mean, radius_std, angle_std)
    in_maps = []
    for k in range(NCORES):
        s = sc[k]
        scal = np.concatenate(
            [s["a2"], s["s2"], s["sr"], s["br"], s["lx"]], axis=1)
        cst = np.ascontiguousarray(
            np.concatenate([fld2, ident, scal], axis=1))
        in_maps.append({"cst": cst})
    res = run_bass_kernel_spmd(nc, in_maps, core_ids=list(range(NCORES)))
    out = np.empty((B, C, H, W), dtype=np.float32)
    for k in range(NCORES):
        r = res.results[k]["out"].reshape(B, CL, H, W)
        out[:, k * CL : (k + 1) * CL] = r
    return out



# revision 2
# speedup vs baseline: 1.1987x; 1.1987x over previous
"""LongRangeProj Bass kernel v2 for TRN2 (8 cores, channel-sharded).

out[b,c,h,w] = max_o x[b,c,o] * exp(-( inv2rv*(fn-|rm|)^2 + inv2av*wrap^2 ))
            = exp(-min_o( sqa + rdn - lx )),  lx = ln x

Per core: CL=8 channels, partitions = 2 batches x 64 origins, free = 4096 px.
Per channel:
  wu   = add_range_wrap(V16 + a2w)              [DVE custom, fp16, exact wrap]
  sqa  = Square(s2 * wu)                        [ACT, fp16 out]
  rdn  = Square(sr * FN32 + br)                 [ACT, fp16 out]
  psum = sqa^T + rdn^T + W10^T @ R10            [PE fp16 matmuls, fp32 PSUM]
         (rank-10 adds -lx (hi+lo) and the origin-pixel mask correction
          -sqa_diag at the diagonal cells)
  o_t  = min over origins                       [DVE tensor_reduce from PSUM]
  out  = exp(-o_t)                              [ACT], PE-transpose, 2 DMAs
"""

import numpy as np
from contextlib import ExitStack

B, C, NH, NW, H, W = 2, 64, 8, 8, 64, 64
STRIDE = 8
NCORES = 8
CL = C // NCORES          # 8 channels per core
HW = H * W                # 4096
NO = NH * NW              # 64 origins
NBLK = HW // 128          # 32 pixel blocks
CHUNK = 2048              # psum chunk (4 banks)
NCHUNK = HW // CHUNK      # 4
BPC = CHUNK // 128        # 8 blocks per chunk
TWO_PI = 2.0 * np.pi

_built = {}


def _host_fields():
    """V (theta/2pi) fp16 and FN fp32, [128, HW] (dup over 2 batches)."""
    oy = np.arange(NH, dtype=np.float64) * STRIDE
    ox = np.arange(NW, dtype=np.float64) * STRIDE
    yg = np.arange(H, dtype=np.float64)
    xg = np.arange(W, dtype=np.float64)
    fy = yg[None, :] - oy[:, None]                      # [NH, H]
    fx = xg[None, :] - ox[:, None]                      # [NW, W]
    FY = np.broadcast_to(fy[:, None, :, None], (NH, NW, H, W))
    FX = np.broadcast_to(fx[None, :, None, :], (NH, NW, H, W))
    fn = np.sqrt(FX * FX + FY * FY)
    v = np.arctan2(FY, FX) / TWO_PI                     # [-0.5, 0.5]
    v = v.reshape(NO, HW)
    fn = fn.reshape(NO, HW)
    V16 = np.ascontiguousarray(np.concatenate([v, v], 0).astype(np.float16))
    FN32 = np.ascontiguousarray(np.concatenate([fn, fn], 0).astype(np.float32))
    return V16, FN32


def _host_consts():
    V16, FN32 = _host_fields()
    I16 = np.eye(128, dtype=np.float16)
    # W10: rows 0,1 = ones (lx hi/lo); rows 2+j = one-hot at partition 8j
    W10 = np.zeros((10, 128), dtype=np.float16)
    W10[0, :] = 1.0
    W10[1, :] = 1.0
    for j in range(NW):
        W10[2 + j, 8 * j] = 1.0
    return V16, FN32, I16, W10


def _host_scalars(x, radius_mean, angle_mean, radius_std, angle_std):
    """Per-core SCAL [128, 4*CL] fp32 and R10 [10, CL*HW] fp16."""
    inv2rv = 1.0 / (2.0 * (radius_std.astype(np.float64) ** 2 + 0.01))   # [C]
    inv2av = 1.0 / (2.0 * (angle_std.astype(np.float64) ** 2 + 0.0001))  # [C]
    s2c = TWO_PI * np.sqrt(inv2av)                                        # [C]
    src = np.sqrt(inv2rv)                                                 # [C]
    rm = np.abs(radius_mean.astype(np.float64)).reshape(B, C, NO)
    am = angle_mean.astype(np.float64).reshape(B, C, NO)
    xx = np.maximum(x.astype(np.float64).reshape(B, C, NO), 1e-35)
    lx = np.log(xx)                                                       # [B,C,NO]
    a2 = -am / TWO_PI
    a2w = a2 - np.round(a2)                                               # [-0.5,0.5]
    # diagonal correction: -sqa at origin pixel (V=0 there -> wu = a2w)
    sqad = (s2c[None, :, None] * a2w) ** 2                                # [B,C,NO]

    per_core = []
    for k in range(NCORES):
        cs = np.arange(k * CL, (k + 1) * CL)
        scal = np.zeros((128, 4 * CL), np.float32)
        r10 = np.zeros((10, CL, NCHUNK, BPC, 128), np.float32)
        for t, c in enumerate(cs):
            for b in range(B):
                p = slice(b * NO, (b + 1) * NO)
                scal[p, 0 * CL + t] = a2w[b, c]
                scal[p, 1 * CL + t] = s2c[c]
                scal[p, 2 * CL + t] = src[c]
                scal[p, 3 * CL + t] = -rm[b, c] * src[c]
            # R10 columns: col = b*64 + (i*8+j) within each 128-block
            lxh = lx[:, c, :].astype(np.float16).astype(np.float64)       # [B,NO]
            lxl = lx[:, c, :] - lxh
            col_lxh = np.concatenate([-lxh[0], -lxh[1]])                  # [128]
            col_lxl = np.concatenate([-lxl[0], -lxl[1]])
            r10[0, t, :, :, :] = col_lxh[None, None, :]
            r10[1, t, :, :, :] = col_lxl[None, None, :]
            # delta rows: for global block g = 4i (+0..3? g=4i only when
            # 8j<128 always true): block g holds pixels [128g,128g+128) and
            # origin (i,j) diag pixel is global 512i+8j -> g=4i, p=8j.
            for i in range(NH):
                g = 4 * i
                ch_, bl_ = divmod(g, BPC)
                for j in range(NW):
                    for b in range(B):
                        col = b * NO + i * NW + j
                        r10[2 + j, t, ch_, bl_, col] = -sqad[b, c, i * NW + j]
        per_core.append((scal, np.ascontiguousarray(
            r10.reshape(10, CL * HW).astype(np.float16))))
    return per_core


def _build_bass():
    import concourse.bacc as bacc
    import concourse.tile as tile
    import concourse.mybir as mybir

    f32 = mybir.dt.float32
    f16 = mybir.dt.float16
    AF = mybir.ActivationFunctionType
    OP = mybir.AluOpType
    AX = mybir.AxisListType

    nc = bacc.Bacc("TRN2", target_bir_lowering=False)
    V16d = nc.dram_tensor("V16", [128, HW], f16, kind="ExternalInput")
    FN32d = nc.dram_tensor("FN32", [128, HW], f32, kind="ExternalInput")
    I16d = nc.dram_tensor("I16", [128, 128], f16, kind="ExternalInput")
    W10d = nc.dram_tensor("W10", [10, 128], f16, kind="ExternalInput")
    SCALd = nc.dram_tensor("SCAL", [128, 4 * CL], f32, kind="ExternalInput")
    R10d = nc.dram_tensor("R10", [10, CL * HW], f16, kind="ExternalInput")
    out_d = nc.dram_tensor("out", [2 * CL, HW], f32, kind="ExternalOutput")

    with ExitStack() as ctx:
        tc = ctx.enter_context(tile.TileContext(nc))
        cpool = ctx.enter_context(tc.tile_pool(name="const", bufs=1))
        wpool = ctx.enter_context(tc.tile_pool(name="work", bufs=2))
        opool = ctx.enter_context(tc.tile_pool(name="outp", bufs=2))
        psum = ctx.enter_context(tc.tile_pool(name="psum", bufs=2, space="PSUM"))

        V16 = cpool.tile([128, HW], f16, tag="V16", name="V16")
        FN32 = cpool.tile([128, HW], f32, tag="FN32", name="FN32")
        I16 = cpool.tile([128, 128], f16, tag="I16", name="I16")
        W10 = cpool.tile([10, 128], f16, tag="W10", name="W10")
        SCAL = cpool.tile([128, 4 * CL], f32, tag="SCAL", name="SCAL")
        R10 = cpool.tile([10, CL * HW], f16, tag="R10", name="R10")
        # ordered so ARW ch0 (SCAL+V16) then rdn (FN32) start earliest
        nc.sync.dma_start(SCAL[:, :], SCALd[:, :])
        nc.sync.dma_start(V16[:, :], V16d[:, :])
        nc.sync.dma_start(I16[:, :], I16d[:, :])
        nc.sync.dma_start(W10[:, :], W10d[:, :])
        nc.sync.dma_start(FN32[:, :], FN32d[:, :])
        nc.sync.dma_start(R10[:, :], R10d[:, :])

        oe_all = opool.tile([128, CL, 2, 32], f16, tag="oe", name="oe")
        OF = opool.tile([64, CL, 128], f32, tag="OF", name="OF")

        for it in range(CL):
            a2w = SCAL[:, 0 * CL + it : 0 * CL + it + 1]
            s2 = SCAL[:, 1 * CL + it : 1 * CL + it + 1]
            sr = SCAL[:, 2 * CL + it : 2 * CL + it + 1]
            br = SCAL[:, 3 * CL + it : 3 * CL + it + 1]

            wu = wpool.tile([128, HW], f16, tag="wu", name="wu")
            nc.vector.add_range_wrap(wu[:], V16[:], a2w, 0.5, 1.0)
            sqa = wpool.tile([128, HW], f16, tag="sqa", name="sqa")
            nc.scalar.activation(sqa[:], wu[:], AF.Square, scale=s2)
            rdn = wpool.tile([128, HW], f16, tag="rdn", name="rdn")
            nc.scalar.activation(rdn[:], FN32[:], AF.Square, scale=sr, bias=br)

            o_t = wpool.tile([128, NBLK, 2], f32, tag="o_t", name="o_t")
            for chn in range(NCHUNK):
                P = psum.tile([128, CHUNK], f32, tag="P", name="P")
                # rank-10: -lx (hi+lo) + diagonal corrections
                for seg in range(CHUNK // 512):
                    c0 = chn * CHUNK + seg * 512
                    nc.tensor.matmul(
                        P[:, seg * 512 : (seg + 1) * 512],
                        W10[:, :], R10[:, it * HW + c0 : it * HW + c0 + 512],
                        start=True, stop=False, skip_group_check=True)
                for blk in range(BPC):
                    g = chn * BPC + blk
                    sl = slice(g * 128, (g + 1) * 128)
                    po = slice(blk * 128, (blk + 1) * 128)
                    nc.tensor.matmul(P[:, po], sqa[:, sl], I16[:],
                                     start=False, stop=False,
                                     skip_group_check=True)
                    nc.tensor.matmul(P[:, po], rdn[:, sl], I16[:],
                                     start=False, stop=True,
                                     skip_group_check=True)
                nc.vector.tensor_reduce(
                    o_t[:, chn * BPC : (chn + 1) * BPC, :],
                    P[:].rearrange("p (l r o) -> p l r o", l=BPC, r=2, o=64),
                    axis=AX.X, op=OP.min)
            # exp(-m), pair-major columns: oe_all[:, it, pair, blk]
            nc.scalar.activation(
                oe_all[:, it, :, :].rearrange("p a b -> p b a"),
                o_t[:, :, :], AF.Exp, scale=-1.0)

        # batched epilogue: transpose all channels' [128, 64] -> [64, CL*128]
        EP = psum.tile([128, CHUNK], f32, tag="P", name="EP")
        for it in range(CL):
            nc.tensor.matmul(EP[0:64, it * 128 : (it + 1) * 128],
                             oe_all[:, it, :, :], I16[:],
                             start=True, stop=True, skip_group_check=True)
        nc.scalar.copy(OF[:, :, :],
                       EP[0:64, 0 : CL * 128].rearrange(
                           "p (i e) -> p i e", i=CL))
        for pair in range(2):
            nc.sync.dma_start(
                out_d[pair * CL : (pair + 1) * CL, :].rearrange(
                    "it (blk e) -> blk it e", blk=32),
                OF[pair * 32 : (pair + 1) * 32, :, :])
    nc.finalize()
    return nc


def kernel(x, radius_mean, angle_mean, radius_std, angle_std):
    from concourse.bass_utils import run_bass_kernel_spmd

    if "nc" not in _built:
        _built["nc"] = _build_bass()
        _built["consts"] = _host_consts()
    nc = _built["nc"]
    V16, FN32, I16, W10 = _built["consts"]
    sc = _host_scalars(x, radius_mean, angle_mean, radius_std, angle_std)
    in_maps = []
    for k in range(NCORES):
        scal, r10 = sc[k]
        in_maps.append({"V16": V16, "FN32": FN32, "I16": I16, "W10": W10,
                        "SCAL": scal, "R10": r10})
    res = run_bass_kernel_spmd(nc, in_maps, core_ids=list(range(NCORES)))
    out = np.empty((B, C, H, W), dtype=np.float32)
    for k in range(NCORES):
        r = res.results[k]["out"].reshape(B, CL, H, W)
        out[:, k * CL : (k + 1) * CL] = r
    return out


# revision 3
# speedup vs baseline: 1.2367x; 1.0317x over previous
"""LongRangeProj Bass kernel v2 for TRN2 (8 cores, channel-sharded).

out[b,c,h,w] = max_o x[b,c,o] * exp(-( inv2rv*(fn-|rm|)^2 + inv2av*wrap^2 ))
            = exp(-min_o( sqa + rdn - lx )),  lx = ln x

Per core: CL=8 channels, partitions = 2 batches x 64 origins, free = 4096 px.
Per channel:
  wu   = add_range_wrap(V16 + a2w)              [DVE custom, fp16, exact wrap]
  sqa  = Square(s2 * wu)                        [ACT, fp16 out]
  rdn  = Square(sr * FN32 + br)                 [ACT, fp16 out]
  psum = sqa^T + rdn^T + W10^T @ R10            [PE fp16 matmuls, fp32 PSUM]
         (rank-10 adds -lx (hi+lo) and the origin-pixel mask correction
          -sqa_diag at the diagonal cells)
  o_t  = min over origins                       [DVE tensor_reduce from PSUM]
  out  = exp(-o_t)                              [ACT], PE-transpose, 2 DMAs
"""

import numpy as np
from contextlib import ExitStack

B, C, NH, NW, H, W = 2, 64, 8, 8, 64, 64
STRIDE = 8
NCORES = 8
CL = C // NCORES          # 8 channels per core
HW = H * W                # 4096
NO = NH * NW              # 64 origins
NBLK = HW // 128          # 32 pixel blocks
CHUNK = 2048              # psum chunk (4 banks)
NCHUNK = HW // CHUNK      # 4
BPC = CHUNK // 128        # 8 blocks per chunk
TWO_PI = 2.0 * np.pi

_built = {}


def _host_fields():
    """V (theta/2pi) fp16 and FN fp32, [128, HW] (dup over 2 batches)."""
    oy = np.arange(NH, dtype=np.float64) * STRIDE
    ox = np.arange(NW, dtype=np.float64) * STRIDE
    yg = np.arange(H, dtype=np.float64)
    xg = np.arange(W, dtype=np.float64)
    fy = yg[None, :] - oy[:, None]                      # [NH, H]
    fx = xg[None, :] - ox[:, None]                      # [NW, W]
    FY = np.broadcast_to(fy[:, None, :, None], (NH, NW, H, W))
    FX = np.broadcast_to(fx[None, :, None, :], (NH, NW, H, W))
    fn = np.sqrt(FX * FX + FY * FY)
    v = np.arctan2(FY, FX) / TWO_PI                     # [-0.5, 0.5]
    v = v.reshape(NO, HW)
    fn = fn.reshape(NO, HW)
    V16 = np.ascontiguousarray(np.concatenate([v, v], 0).astype(np.float16))
    FN32 = np.ascontiguousarray(np.concatenate([fn, fn], 0).astype(np.float32))
    return V16, FN32


def _host_consts():
    V16, FN32 = _host_fields()
    I16 = np.eye(128, dtype=np.float16)
    # W10: rows 0,1 = ones (lx hi/lo); rows 2+j = one-hot at partition 8j
    W10 = np.zeros((10, 128), dtype=np.float16)
    W10[0, :] = 1.0
    W10[1, :] = 1.0
    for j in range(NW):
        W10[2 + j, 8 * j] = 1.0
    return V16, FN32, I16, W10


def _host_scalars(x, radius_mean, angle_mean, radius_std, angle_std):
    """Per-core SCAL [128, 4*CL] fp32 and R10 [10, CL*HW] fp16."""
    inv2rv = 1.0 / (2.0 * (radius_std.astype(np.float64) ** 2 + 0.01))   # [C]
    inv2av = 1.0 / (2.0 * (angle_std.astype(np.float64) ** 2 + 0.0001))  # [C]
    s2c = TWO_PI * np.sqrt(inv2av)                                        # [C]
    src = np.sqrt(inv2rv)                                                 # [C]
    rm = np.abs(radius_mean.astype(np.float64)).reshape(B, C, NO)
    am = angle_mean.astype(np.float64).reshape(B, C, NO)
    xx = np.maximum(x.astype(np.float64).reshape(B, C, NO), 1e-35)
    lx = np.log(xx)                                                       # [B,C,NO]
    a2 = -am / TWO_PI
    a2w = a2 - np.round(a2)                                               # [-0.5,0.5]
    # diagonal correction: -sqa at origin pixel (V=0 there -> wu = a2w)
    sqad = (s2c[None, :, None] * a2w) ** 2                                # [B,C,NO]

    per_core = []
    for k in range(NCORES):
        cs = np.arange(k * CL, (k + 1) * CL)
        scal = np.zeros((128, 4 * CL), np.float32)
        r10 = np.zeros((10, CL, NCHUNK, BPC, 128), np.float32)
        for t, c in enumerate(cs):
            for b in range(B):
                p = slice(b * NO, (b + 1) * NO)
                scal[p, 0 * CL + t] = a2w[b, c]
                scal[p, 1 * CL + t] = s2c[c]
                scal[p, 2 * CL + t] = src[c]
                scal[p, 3 * CL + t] = -rm[b, c] * src[c]
            # R10 columns: col = b*64 + (i*8+j) within each 128-block
            lxh = lx[:, c, :].astype(np.float16).astype(np.float64)       # [B,NO]
            lxl = lx[:, c, :] - lxh
            col_lxh = np.concatenate([-lxh[0], -lxh[1]])                  # [128]
            col_lxl = np.concatenate([-lxl[0], -lxl[1]])
            r10[0, t, :, :, :] = col_lxh[None, None, :]
            r10[1, t, :, :, :] = col_lxl[None, None, :]
            # delta rows: for global block g = 4i (+0..3? g=4i only when
            # 8j<128 always true): block g holds pixels [128g,128g+128) and
            # origin (i,j) diag pixel is global 512i+8j -> g=4i, p=8j.
            for i in range(NH):
                g = 4 * i
                ch_, bl_ = divmod(g, BPC)
                for j in range(NW):
                    for b in range(B):
                        col = b * NO + i * NW + j
                        r10[2 + j, t, ch_, bl_, col] = -sqad[b, c, i * NW + j]
        per_core.append((scal, np.ascontiguousarray(
            r10.reshape(10, CL * HW).astype(np.float16))))
    return per_core


def _build_bass():
    import concourse.bacc as bacc
    import concourse.tile as tile
    import concourse.mybir as mybir

    f32 = mybir.dt.float32
    f16 = mybir.dt.float16
    AF = mybir.ActivationFunctionType
    OP = mybir.AluOpType
    AX = mybir.AxisListType

    nc = bacc.Bacc("TRN2", target_bir_lowering=False)
    VI16d = nc.dram_tensor("VI16", [128, HW + 128], f16, kind="ExternalInput")
    FN32d = nc.dram_tensor("FN32", [128, HW], f32, kind="ExternalInput")
    SCALd = nc.dram_tensor("SCAL", [128, 4 * CL], f32, kind="ExternalInput")
    WRd = nc.dram_tensor("WR", [10, 128 + CL * HW], f16, kind="ExternalInput")
    out_d = nc.dram_tensor("out", [2 * CL, HW], f32, kind="ExternalOutput")

    with ExitStack() as ctx:
        tc = ctx.enter_context(tile.TileContext(nc))
        cpool = ctx.enter_context(tc.tile_pool(name="const", bufs=1))
        wpool = ctx.enter_context(tc.tile_pool(name="work", bufs=2))
        opool = ctx.enter_context(tc.tile_pool(name="outp", bufs=2))
        psum = ctx.enter_context(tc.tile_pool(name="psum", bufs=2, space="PSUM"))

        VI16 = cpool.tile([128, HW + 128], f16, tag="VI16", name="VI16")
        FN32 = cpool.tile([128, HW], f32, tag="FN32", name="FN32")
        SCAL = cpool.tile([128, 4 * CL], f32, tag="SCAL", name="SCAL")
        WR = cpool.tile([10, 128 + CL * HW], f16, tag="WR", name="WR")
        V16 = VI16[:, 0:HW]
        I16 = VI16[:, HW : HW + 128]
        W10 = WR[:, 0:128]
        # ordered so ARW ch0 (SCAL+V16) then rdn (FN32) start earliest
        nc.sync.dma_start(SCAL[:, :], SCALd[:, :])
        nc.sync.dma_start(VI16[:, :], VI16d[:, :])
        nc.sync.dma_start(FN32[:, :], FN32d[:, :])
        nc.sync.dma_start(WR[:, :], WRd[:, :])

        oe_all = opool.tile([128, CL, 2, 32], f16, tag="oe", name="oe")
        OF = opool.tile([64, CL, 128], f32, tag="OF", name="OF")

        for it in range(CL):
            a2w = SCAL[:, 0 * CL + it : 0 * CL + it + 1]
            s2 = SCAL[:, 1 * CL + it : 1 * CL + it + 1]
            sr = SCAL[:, 2 * CL + it : 2 * CL + it + 1]
            br = SCAL[:, 3 * CL + it : 3 * CL + it + 1]

            wu = wpool.tile([128, HW], f16, tag="wu", name="wu")
            nc.vector.add_range_wrap(wu[:], V16, a2w, 0.5, 1.0)
            sqa = wpool.tile([128, HW], f16, tag="sqa", name="sqa")
            nc.scalar.activation(sqa[:], wu[:], AF.Square, scale=s2)
            rdn = wpool.tile([128, HW], f16, tag="rdn", name="rdn")
            nc.scalar.activation(rdn[:], FN32[:], AF.Square, scale=sr, bias=br)

            o_t = wpool.tile([128, NBLK, 2], f32, tag="o_t", name="o_t")
            for chn in range(NCHUNK):
                P = psum.tile([128, CHUNK], f32, tag="P", name="P")
                # rank-10: -lx (hi+lo) + diagonal corrections
                for seg in range(CHUNK // 512):
                    c0 = chn * CHUNK + seg * 512
                    nc.tensor.matmul(
                        P[:, seg * 512 : (seg + 1) * 512],
                        W10, WR[:, 128 + it * HW + c0 : 128 + it * HW + c0 + 512],
                        start=True, stop=False, skip_group_check=True)
                for blk in range(BPC):
                    g = chn * BPC + blk
                    sl = slice(g * 128, (g + 1) * 128)
                    po = slice(blk * 128, (blk + 1) * 128)
                    nc.tensor.matmul(P[:, po], sqa[:, sl], I16,
                                     start=False, stop=False,
                                     skip_group_check=True)
                    nc.tensor.matmul(P[:, po], rdn[:, sl], I16,
                                     start=False, stop=True,
                                     skip_group_check=True)
                nc.vector.tensor_reduce(
                    o_t[:, chn * BPC : (chn + 1) * BPC, :],
                    P[:].rearrange("p (l r o) -> p l r o", l=BPC, r=2, o=64),
                    axis=AX.X, op=OP.min)
            # exp(-m), pair-major columns: oe_all[:, it, pair, blk]
            nc.scalar.activation(
                oe_all[:, it, :, :].rearrange("p a b -> p b a"),
                o_t[:, :, :], AF.Exp, scale=-1.0)

        # batched epilogue: transpose all channels' [128, 64] -> [64, CL*128]
        EP = psum.tile([128, CHUNK], f32, tag="P", name="EP")
        for it in range(CL):
            nc.tensor.matmul(EP[0:64, it * 128 : (it + 1) * 128],
                             oe_all[:, it, :, :], I16,
                             start=True, stop=True, skip_group_check=True)
        nc.scalar.copy(OF[:, :, :],
                       EP[0:64, 0 : CL * 128].rearrange(
                           "p (i e) -> p i e", i=CL))
        for pair in range(2):
            nc.sync.dma_start(
                out_d[pair * CL : (pair + 1) * CL, :].rearrange(
                    "it (blk e) -> blk it e", blk=32),
                OF[pair * 32 : (pair + 1) * 32, :, :])
    nc.finalize()
    return nc


def kernel(x, radius_mean, angle_mean, radius_std, angle_std):
    from concourse.bass_utils import run_bass_kernel_spmd

    if "nc" not in _built:
        _built["nc"] = _build_bass()
        _built["consts"] = _host_consts()
    nc = _built["nc"]
    V16, FN32, I16, W10 = _built["consts"]
    VI16 = np.ascontiguousarray(np.concatenate([V16, I16], axis=1))
    sc = _host_scalars(x, radius_mean, angle_mean, radius_std, angle_std)
    in_maps = []
    for k in range(NCORES):
        scal, r10 = sc[k]
        wr = np.ascontiguousarray(np.concatenate([W10, r10], axis=1))
        in_maps.append({"VI16": VI16, "FN32": FN32, "SCAL": scal, "WR": wr})
    res = run_bass_kernel_spmd(nc, in_maps, core_ids=list(range(NCORES)))
    out = np.empty((B, C, H, W), dtype=np.float32)
    for k in range(NCORES):
        r = res.results[k]["out"].reshape(B, CL, H, W)
        out[:, k * CL : (k + 1) * CL] = r
    return out
